# revision 5
# baseline (speedup 1.0000x reference)
"""DbrxExpertGLU (single-expert SwiGLU MLP) Trainium2 kernel.

  down = (silu(x @ w1.T) * (x @ v1.T)) @ w2
  x: [4096, 4096] f32, w1/v1/w2: [14336, 4096] f32 -> out [4096, 4096] f32

Strategy (8 NeuronCores, tensor-parallel over ffn dim per the expert-TP
hint): shard F=14336 into 8 x 1792. Each core computes gate/up/inter for
its F-shard and a partial down [4096, 4096]; the host sums the 8 partials.

All three matmuls run in fp8 e4m3 DoubleRow mode (double-pumped PE rows:
each instruction contracts 256 rows at 0.5 cycles per output element).
To stay within the accuracy budget each operand is Dekker-split into
hi + lo e4m3 tensors and the three significant cross products
(hi*hi + hi*lo + lo*hi) are accumulated in PSUM, i.e. 3 double-pumped
passes replace 2 bf16-rate passes per 256 contraction rows -> 0.75x the
bf16 PE time with ~3e-3 relative error. Weights/x are split on the host
(with power-of-2 pre-scales to keep values in e4m3's normal range; the
scales are divided back out inside ACT ops). The SwiGLU intermediate is
split on-device (ACT copy for hi, one DVE scalar_tensor_tensor for lo).

Token chunks of 512 are processed in pairs sharing one weight pass so
weight DMA traffic halves; partial outputs return as bf16.
"""

import os
import subprocess
import sys
import tempfile
import time
from contextlib import ExitStack

import numpy as np
import ml_dtypes

import concourse.bass as bass
import concourse.mybir as mybir
import concourse.tile as tile
from concourse import bacc
from concourse.bass_utils import run_bass_kernel_spmd

BF16 = mybir.dt.bfloat16
F32 = mybir.dt.float32
FP8 = mybir.dt.float8e4
E4 = ml_dtypes.float8_e4m3
DR = mybir.MatmulPerfMode.DoubleRow
AF = mybir.ActivationFunctionType
ALU = mybir.AluOpType

T, H, F = 4096, 4096, 14336
N_CORES = 8
FS = F // N_CORES            # 1792 ffn rows per core
TC = 512                     # token chunk (= matmul moving dim)
NT = T // TC                 # 8 token chunks, processed in 4 pairs
C = H // 256                 # 16 k-chunks of 256 for gate/up contraction
FBN = FS // 128              # 14 f-blocks per core
JN = FS // 256               # 7 k-chunks of 256 for down contraction
HB = H // 128                # 32 h-blocks

# power-of-2 pre-scales keeping every fp8 operand in e4m3's normal range
SX, S1, SV, S2 = 4.0, 64.0, 64.0, 64.0
SIG_SCALE = 1.0 / (SX * S1)          # sigmoid arg: true gate
CAST_SCALE = 1.0 / (SX * SX * S1 * SV)  # raw psum product -> true inter
OUT_SCALE = 1.0 / S2                 # down psum -> true partial

_NC_CACHE = []


def _build():
    nc = bacc.Bacc("TRN2", target_bir_lowering=False, debug=False)

    xh_d = nc.dram_tensor("xh", [NT, 128, C, 2, TC], FP8, kind="ExternalInput").ap()
    xl_d = nc.dram_tensor("xl", [NT, 128, C, 2, TC], FP8, kind="ExternalInput").ap()
    w1h_d = nc.dram_tensor("w1h", [FBN, 128, C, 2, 128], FP8, kind="ExternalInput").ap()
    w1l_d = nc.dram_tensor("w1l", [FBN, 128, C, 2, 128], FP8, kind="ExternalInput").ap()
    v1h_d = nc.dram_tensor("v1h", [FBN, 128, C, 2, 128], FP8, kind="ExternalInput").ap()
    v1l_d = nc.dram_tensor("v1l", [FBN, 128, C, 2, 128], FP8, kind="ExternalInput").ap()
    w2h_d = nc.dram_tensor("w2h", [HB, 128, JN, 2, 128], FP8, kind="ExternalInput").ap()
    w2l_d = nc.dram_tensor("w2l", [HB, 128, JN, 2, 128], FP8, kind="ExternalInput").ap()
    out_d = nc.dram_tensor("out", [H, T], BF16, kind="ExternalOutput").ap()

    with tile.TileContext(nc) as tc, ExitStack() as ctx:
        xh_pool = ctx.enter_context(tc.tile_pool(name="xh", bufs=2))
        xl_pool = ctx.enter_context(tc.tile_pool(name="xl", bufs=2))
        w1h_pool = ctx.enter_context(tc.tile_pool(name="w1h", bufs=3))
        w1l_pool = ctx.enter_context(tc.tile_pool(name="w1l", bufs=3))
        v1h_pool = ctx.enter_context(tc.tile_pool(name="v1h", bufs=3))
        v1l_pool = ctx.enter_context(tc.tile_pool(name="v1l", bufs=3))
        w2h_pool = ctx.enter_context(tc.tile_pool(name="w2h", bufs=2))
        w2l_pool = ctx.enter_context(tc.tile_pool(name="w2l", bufs=2))
        ihi_pool = ctx.enter_context(tc.tile_pool(name="ihi", bufs=2))
        ilo_pool = ctx.enter_context(tc.tile_pool(name="ilo", bufs=2))
        sg_pool = ctx.enter_context(tc.tile_pool(name="sg", bufs=2))
        sl_pool = ctx.enter_context(tc.tile_pool(name="sl", bufs=2))
        raw_pool = ctx.enter_context(tc.tile_pool(name="raw", bufs=2))
        out_pool = ctx.enter_context(tc.tile_pool(name="outp", bufs=4))
        pg_pool = ctx.enter_context(tc.tile_pool(name="pg", bufs=2, space="PSUM"))
        pu_pool = ctx.enter_context(tc.tile_pool(name="pu", bufs=2, space="PSUM"))
        pd_pool = ctx.enter_context(tc.tile_pool(name="pd", bufs=3, space="PSUM"))

        def x_dma(tile_, src, tt, c0, c1):
            dst = tile_[:].rearrange("p (c i t) -> p c i t", c=C, i=2)
            nc.scalar.dma_start(out=dst[:, c0:c1], in_=src[tt, :, c0:c1])

        def alloc_x():
            xh_t = xh_pool.tile([128, C * 2 * TC], FP8, name="xht")
            xl_t = xl_pool.tile([128, C * 2 * TC], FP8, name="xlt")
            return xh_t, xl_t

        def load_x_full(xt_pair, tt):
            x_dma(xt_pair[0], xh_d, tt, 0, C)
            x_dma(xt_pair[1], xl_d, tt, 0, C)

        W_SRCS = (w1h_d, w1l_d, v1h_d, v1l_d)
        W_POOLS = (w1h_pool, w1l_pool, v1h_pool, v1l_pool)

        def alloc_w1():
            return tuple(p.tile([128, C * 2 * 128], FP8, name="wt") for p in W_POOLS)

        def w_dma(tile_, src, fb, c0=0, c1=C):
            dst = tile_[:].rearrange("p (c i m) -> p c i m", c=C, i=2)
            nc.sync.dma_start(out=dst[:, c0:c1], in_=src[fb, :, c0:c1])

        def load_w1_full(tiles, fb):
            for t_, s_ in zip(tiles, W_SRCS):
                w_dma(t_, s_, fb)

        # pass order inside a psum group: hi*hi, lo*hi, hi*lo -- the x_lo
        # tensor is only needed two-thirds into the group, easing the
        # pair-0 DMA-starved head
        def mm_group(psum, wh, wl, xh, xl):
            n = 3 * C
            k = 0
            for wt, xv in ((wh, xh), (wl, xh), (wh, xl)):
                for c in range(C):
                    nc.tensor.matmul(psum[:], wt[:, c], xv[:, c],
                                     start=(k == 0), stop=(k == n - 1),
                                     perf_mode=DR)
                    k += 1

        w_next = None
        xt_next = None
        for pr in range(NT // 2):
            t0, t1 = 2 * pr, 2 * pr + 1
            if pr == 0:
                # staged fine-grained first loads, priority-ordered for the
                # shared DMA pipe: x_hi[t0], w1[fb0], x_lo[t0], v1[fb0],
                # x[t1], w[fb1]
                xt = {t0: alloc_x(), t1: alloc_x()}
                w_first = {0: alloc_w1(), 1: alloc_w1()}
                bounds = [0, 1, 2, 4, 8, 12, C]
                for c0, c1 in zip(bounds, bounds[1:]):
                    x_dma(xt[t0][0], xh_d, t0, c0, c1)
                for c0 in range(0, C, 4):
                    w_dma(w_first[0][0], w1h_d, 0, c0, c0 + 4)
                    w_dma(w_first[0][1], w1l_d, 0, c0, c0 + 4)
                for c0, c1 in zip(bounds, bounds[1:]):
                    x_dma(xt[t0][1], xl_d, t0, c0, c1)
                for c0 in range(0, C, 8):
                    w_dma(w_first[0][2], v1h_d, 0, c0, c0 + 8)
                    w_dma(w_first[0][3], v1l_d, 0, c0, c0 + 8)
                for c0 in range(0, C, 4):
                    x_dma(xt[t1][0], xh_d, t1, c0, c0 + 4)
                load_w1_full(w_first[1], 1)
                for c0 in range(0, C, 4):
                    x_dma(xt[t1][1], xl_d, t1, c0, c0 + 4)
            else:
                xt = {t0: xt_next[0], t1: xt_next[1]}
                w_first = w_next

            ihi = {tt: ihi_pool.tile([128, FBN * TC], FP8, name="ihit")
                   for tt in (t0, t1)}
            ilo = {tt: ilo_pool.tile([128, FBN * TC], FP8, name="ilot")
                   for tt in (t0, t1)}

            # phase A: gate/up -> inter (hi/lo e4m3), per 128-row f-block
            for fb in range(FBN):
                if fb in (0, 1):
                    w1h_t, w1l_t, v1h_t, v1l_t = w_first[fb]
                else:
                    tiles = alloc_w1()
                    load_w1_full(tiles, fb)
                    w1h_t, w1l_t, v1h_t, v1l_t = tiles
                w1hv = w1h_t[:].rearrange("p (c i m) -> p c i m", c=C, i=2)
                w1lv = w1l_t[:].rearrange("p (c i m) -> p c i m", c=C, i=2)
                v1hv = v1h_t[:].rearrange("p (c i m) -> p c i m", c=C, i=2)
                v1lv = v1l_t[:].rearrange("p (c i m) -> p c i m", c=C, i=2)
                for tt in (t0, t1):
                    xhv = xt[tt][0][:].rearrange("p (c i t) -> p c i t", c=C, i=2)
                    xlv = xt[tt][1][:].rearrange("p (c i t) -> p c i t", c=C, i=2)
                    pg = pg_pool.tile([128, TC], F32)
                    pu = pu_pool.tile([128, TC], F32)
                    mm_group(pg, w1hv, w1lv, xhv, xlv)
                    mm_group(pu, v1hv, v1lv, xhv, xlv)
                    sg = sg_pool.tile([128, TC], F32)
                    nc.scalar.activation(sg[:], pg[:], AF.Sigmoid, scale=SIG_SCALE)
                    sl = sl_pool.tile([128, TC], F32)
                    nc.vector.tensor_mul(sl[:], sg[:], pg[:])
                    raw = raw_pool.tile([128, TC], F32)
                    nc.vector.tensor_mul(raw[:], sl[:], pu[:])
                    hi_sl = ihi[tt][:, bass.ts(fb, TC)]
                    nc.scalar.activation(hi_sl, raw[:], AF.Copy, scale=CAST_SCALE)
                    nc.vector.scalar_tensor_tensor(
                        ilo[tt][:, bass.ts(fb, TC)], raw[:], CAST_SCALE, hi_sl,
                        op0=ALU.mult, op1=ALU.subtract)

            # next pair's first weights + x, interleaved into phase B's DMA
            # stream (shared-pipe FIFO: w2[hb] must stay just-in-time)
            last = pr == NT // 2 - 1
            prefetch = []
            if not last:
                w_next = {0: alloc_w1(), 1: alloc_w1()}
                xt_next = (alloc_x(), alloc_x())
                na, nb = 2 * pr + 2, 2 * pr + 3
                for fbn in (0, 1):
                    for t_, s_ in zip(w_next[fbn], W_SRCS):
                        prefetch.append(lambda t_=t_, s_=s_, fbn=fbn: w_dma(t_, s_, fbn))
                for q in range(0, C, 4):
                    prefetch.append(lambda q=q: x_dma(xt_next[0][0], xh_d, na, q, q + 4))
                    prefetch.append(lambda q=q: x_dma(xt_next[0][1], xl_d, na, q, q + 4))
                    prefetch.append(lambda q=q: x_dma(xt_next[1][0], xh_d, nb, q, q + 4))
                    prefetch.append(lambda q=q: x_dma(xt_next[1][1], xl_d, nb, q, q + 4))

            # phase B: partial down, per 128-row h-block
            for hb in range(HB):
                w2h_t = w2h_pool.tile([128, JN * 2 * 128], FP8)
                w2l_t = w2l_pool.tile([128, JN * 2 * 128], FP8)
                nc.sync.dma_start(
                    out=w2h_t[:].rearrange("p (j i m) -> p j i m", j=JN, i=2),
                    in_=w2h_d[hb])
                nc.sync.dma_start(
                    out=w2l_t[:].rearrange("p (j i m) -> p j i m", j=JN, i=2),
                    in_=w2l_d[hb])
                if hb >= 2 and prefetch:
                    prefetch.pop(0)()
                w2hv = w2h_t[:].rearrange("p (j i m) -> p j i m", j=JN, i=2)
                w2lv = w2l_t[:].rearrange("p (j i m) -> p j i m", j=JN, i=2)
                for tt in (t0, t1):
                    ihv = ihi[tt][:].rearrange("p (fb t) -> p fb t", fb=FBN)
                    ilv = ilo[tt][:].rearrange("p (fb t) -> p fb t", fb=FBN)
                    last_tile = last and hb == HB - 1 and tt == t1
                    splits = (0, 256, 384, 512) if last_tile else (0, TC)
                    for si in range(len(splits) - 1):
                        c0, c1 = splits[si], splits[si + 1]
                        pd = pd_pool.tile([128, c1 - c0], F32)
                        k = 0
                        for j in range(JN):
                            for wt, iv in ((w2hv, ihv), (w2lv, ihv), (w2hv, ilv)):
                                nc.tensor.matmul(
                                    pd[:], wt[:, j],
                                    iv[:, 2 * j:2 * j + 2, c0:c1],
                                    start=(k == 0), stop=(k == 3 * JN - 1),
                                    perf_mode=DR)
                                k += 1
                        ob = out_pool.tile([128, c1 - c0], BF16)
                        nc.scalar.activation(ob[:], pd[:], AF.Copy, scale=OUT_SCALE)
                        nc.sync.dma_start(
                            out=out_d[hb * 128:(hb + 1) * 128,
                                      tt * TC + c0:tt * TC + c1],
                            in_=ob[:])
            while prefetch:
                prefetch.pop(0)()

    nc.compile()
    return nc


def _split(a):
    hi = a.astype(E4)
    lo = (a - hi.astype(np.float32)).astype(E4)
    return np.ascontiguousarray(hi), np.ascontiguousarray(lo)


def _prep_inputs(x, w1, v1, w2):
    # x[t, h]*SX -> [tci, p(h%128), c(h//256), i((h%256)//128), tt]
    x4 = (x * SX).reshape(NT, TC, C, 2, 128).transpose(0, 4, 2, 3, 1)
    xh, xl = _split(np.ascontiguousarray(x4, dtype=np.float32))
    in_maps = []
    for cid in range(N_CORES):
        sl_ = slice(cid * FS, (cid + 1) * FS)
        # w[f, h]*S -> [fb, p(h%128), c, i, m(f%128)]
        w1s = (w1[sl_] * S1).reshape(FBN, 128, C, 2, 128).transpose(0, 4, 2, 3, 1)
        v1s = (v1[sl_] * SV).reshape(FBN, 128, C, 2, 128).transpose(0, 4, 2, 3, 1)
        # w2[f, h]*S2 -> [hb, p(f%128), j(f//256), i((f%256)//128), m(h%128)]
        w2s = (w2[sl_] * S2).reshape(JN, 2, 128, HB, 128).transpose(3, 2, 0, 1, 4)
        w1h, w1l = _split(np.ascontiguousarray(w1s, dtype=np.float32))
        v1h, v1l = _split(np.ascontiguousarray(v1s, dtype=np.float32))
        w2h, w2l = _split(np.ascontiguousarray(w2s, dtype=np.float32))
        in_maps.append({
            "xh": xh, "xl": xl,
            "w1h": w1h, "w1l": w1l, "v1h": v1h, "v1l": v1l,
            "w2h": w2h, "w2l": w2l,
        })
    return in_maps


def _exec_once(in_maps):
    """One 8-core device execution; returns summed partial [H, T] f32."""
    if not _NC_CACHE:
        _NC_CACHE.append(_build())
    res = run_bass_kernel_spmd(_NC_CACHE[0], in_maps, list(range(N_CORES)))
    acc = res.results[0]["out"].astype(np.float32)
    for c in range(1, N_CORES):
        acc += res.results[c]["out"].astype(np.float32)
    if not np.isfinite(acc).all():
        raise FloatingPointError("non-finite output from device")
    return acc


_IN_KEYS = ("xh", "xl", "w1h", "w1l", "v1h", "v1l", "w2h", "w2l")


def _exec_subprocess(in_maps):
    """Retry path: run the device execution in a fresh process (fresh axon
    client) in case this process's device session is poisoned."""
    base = "/dev/shm" if os.path.isdir("/dev/shm") else None
    with tempfile.TemporaryDirectory(dir=base) as d:
        np.save(os.path.join(d, "xh.npy"), in_maps[0]["xh"].view(np.uint8))
        np.save(os.path.join(d, "xl.npy"), in_maps[0]["xl"].view(np.uint8))
        for c, m in enumerate(in_maps):
            for k in _IN_KEYS[2:]:
                np.save(os.path.join(d, f"{k}_{c}.npy"), m[k].view(np.uint8))
        subprocess.run(
            [sys.executable, os.path.abspath(__file__), "--subproc", d],
            check=True, timeout=1800,
        )
        return np.load(os.path.join(d, "acc.npy"))


def _subproc_main(d):
    xh = np.load(os.path.join(d, "xh.npy")).view(E4)
    xl = np.load(os.path.join(d, "xl.npy")).view(E4)
    in_maps = []
    for c in range(N_CORES):
        m = {"xh": xh, "xl": xl}
        for k in _IN_KEYS[2:]:
            m[k] = np.load(os.path.join(d, f"{k}_{c}.npy")).view(E4)
        in_maps.append(m)
    np.save(os.path.join(d, "acc.npy"), _exec_once(in_maps))


def kernel(x, expert_w1, expert_v1, expert_w2):
    x = np.asarray(x, dtype=np.float32)
    expert_w1 = np.asarray(expert_w1, dtype=np.float32)
    expert_v1 = np.asarray(expert_v1, dtype=np.float32)
    expert_w2 = np.asarray(expert_w2, dtype=np.float32)
    assert x.shape == (T, H) and expert_w1.shape == (F, H)

    in_maps = _prep_inputs(x, expert_w1, expert_v1, expert_w2)

    acc = None
    last_err = None
    for attempt in range(4):
        try:
            if attempt < 2:
                acc = _exec_once(in_maps)
            else:
                acc = _exec_subprocess(in_maps)
            break
        except Exception as e:  # transient device/tunnel errors: retry
            last_err = e
            time.sleep(3.0)
    if acc is None:
        raise last_err
    return np.ascontiguousarray(acc.T)  # [h, t] -> [t, h]


if __name__ == "__main__" and len(sys.argv) == 3 and sys.argv[1] == "--subproc":
    _subproc_main(sys.argv[2])


# revision 6
# speedup vs baseline: 1.0097x; 1.0097x over previous
"""DbrxExpertGLU (single-expert SwiGLU MLP) Trainium2 kernel.

  down = (silu(x @ w1.T) * (x @ v1.T)) @ w2
  x: [4096, 4096] f32, w1/v1/w2: [14336, 4096] f32 -> out [4096, 4096] f32

Strategy (8 NeuronCores, tensor-parallel over ffn dim per the expert-TP
hint): shard F=14336 into 8 x 1792. Each core computes gate/up/inter for
its F-shard and a partial down [4096, 4096]; the host sums the 8 partials.

All three matmuls run in fp8 e4m3 DoubleRow mode (double-pumped PE rows:
each instruction contracts 256 rows at 0.5 cycles per output element).
To stay within the accuracy budget each operand is Dekker-split into
hi + lo e4m3 tensors and the three significant cross products
(hi*hi + hi*lo + lo*hi) are accumulated in PSUM, i.e. 3 double-pumped
passes replace 2 bf16-rate passes per 256 contraction rows -> 0.75x the
bf16 PE time with ~3e-3 relative error. Weights/x are split on the host
(with power-of-2 pre-scales to keep values in e4m3's normal range; the
scales are divided back out inside ACT ops). The SwiGLU intermediate is
split on-device (ACT copy for hi, one DVE scalar_tensor_tensor for lo).

Token chunks of 512 are processed in pairs sharing one weight pass so
weight DMA traffic halves; partial outputs return as bf16.
"""

import os
import subprocess
import sys
import tempfile
import time
from contextlib import ExitStack

import numpy as np
import ml_dtypes

import concourse.bass as bass
import concourse.mybir as mybir
import concourse.tile as tile
from concourse import bacc
from concourse.bass_utils import run_bass_kernel_spmd

BF16 = mybir.dt.bfloat16
F32 = mybir.dt.float32
FP8 = mybir.dt.float8e4
E4 = ml_dtypes.float8_e4m3
DR = mybir.MatmulPerfMode.DoubleRow
AF = mybir.ActivationFunctionType
ALU = mybir.AluOpType

T, H, F = 4096, 4096, 14336
N_CORES = 8
FS = F // N_CORES            # 1792 ffn rows per core
TC = 512                     # token chunk (= matmul moving dim)
NT = T // TC                 # 8 token chunks, processed in 4 pairs
C = H // 256                 # 16 k-chunks of 256 for gate/up contraction
FBN = FS // 128              # 14 f-blocks per core
JN = FS // 256               # 7 k-chunks of 256 for down contraction
HB = H // 128                # 32 h-blocks

# power-of-2 pre-scales keeping every fp8 operand in e4m3's normal range
SX, S1, SV, S2 = 4.0, 64.0, 64.0, 64.0
SIG_SCALE = 1.0 / (SX * S1)          # sigmoid arg: true gate
CAST_SCALE = 1.0 / (SX * SX * S1 * SV)  # raw psum product -> true inter
OUT_SCALE = 1.0 / S2                 # down psum -> true partial

_NC_CACHE = []


def _build():
    nc = bacc.Bacc("TRN2", target_bir_lowering=False, debug=False)

    xh_d = nc.dram_tensor("xh", [NT, 128, C, 2, TC], FP8, kind="ExternalInput").ap()
    xl_d = nc.dram_tensor("xl", [NT, 128, C, 2, TC], FP8, kind="ExternalInput").ap()
    w1h_d = nc.dram_tensor("w1h", [FBN, 128, C, 2, 128], FP8, kind="ExternalInput").ap()
    w1l_d = nc.dram_tensor("w1l", [FBN, 128, C, 2, 128], FP8, kind="ExternalInput").ap()
    v1h_d = nc.dram_tensor("v1h", [FBN, 128, C, 2, 128], FP8, kind="ExternalInput").ap()
    v1l_d = nc.dram_tensor("v1l", [FBN, 128, C, 2, 128], FP8, kind="ExternalInput").ap()
    w2h_d = nc.dram_tensor("w2h", [HB, 128, JN, 2, 128], FP8, kind="ExternalInput").ap()
    w2l_d = nc.dram_tensor("w2l", [HB, 128, JN, 2, 128], FP8, kind="ExternalInput").ap()
    out_d = nc.dram_tensor("out", [H, T], BF16, kind="ExternalOutput").ap()

    with tile.TileContext(nc) as tc, ExitStack() as ctx:
        xh_pool = ctx.enter_context(tc.tile_pool(name="xh", bufs=2))
        xl_pool = ctx.enter_context(tc.tile_pool(name="xl", bufs=2))
        w1h_pool = ctx.enter_context(tc.tile_pool(name="w1h", bufs=3))
        w1l_pool = ctx.enter_context(tc.tile_pool(name="w1l", bufs=3))
        v1h_pool = ctx.enter_context(tc.tile_pool(name="v1h", bufs=3))
        v1l_pool = ctx.enter_context(tc.tile_pool(name="v1l", bufs=3))
        w2h_pool = ctx.enter_context(tc.tile_pool(name="w2h", bufs=2))
        w2l_pool = ctx.enter_context(tc.tile_pool(name="w2l", bufs=2))
        ihi_pool = ctx.enter_context(tc.tile_pool(name="ihi", bufs=2))
        ilo_pool = ctx.enter_context(tc.tile_pool(name="ilo", bufs=2))
        sg_pool = ctx.enter_context(tc.tile_pool(name="sg", bufs=2))
        sl_pool = ctx.enter_context(tc.tile_pool(name="sl", bufs=2))
        raw_pool = ctx.enter_context(tc.tile_pool(name="raw", bufs=2))
        out_pool = ctx.enter_context(tc.tile_pool(name="outp", bufs=4))
        pg_pool = ctx.enter_context(tc.tile_pool(name="pg", bufs=2, space="PSUM"))
        pu_pool = ctx.enter_context(tc.tile_pool(name="pu", bufs=2, space="PSUM"))
        pd_pool = ctx.enter_context(tc.tile_pool(name="pd", bufs=3, space="PSUM"))

        def x_dma(tile_, src, tt, c0, c1):
            dst = tile_[:].rearrange("p (c i t) -> p c i t", c=C, i=2)
            nc.scalar.dma_start(out=dst[:, c0:c1], in_=src[tt, :, c0:c1])

        def alloc_x():
            xh_t = xh_pool.tile([128, C * 2 * TC], FP8, name="xht")
            xl_t = xl_pool.tile([128, C * 2 * TC], FP8, name="xlt")
            return xh_t, xl_t

        def load_x_full(xt_pair, tt):
            x_dma(xt_pair[0], xh_d, tt, 0, C)
            x_dma(xt_pair[1], xl_d, tt, 0, C)

        W_SRCS = (w1h_d, w1l_d, v1h_d, v1l_d)
        W_POOLS = (w1h_pool, w1l_pool, v1h_pool, v1l_pool)

        def alloc_w1():
            return tuple(p.tile([128, C * 2 * 128], FP8, name="wt") for p in W_POOLS)

        def w_dma(tile_, src, fb, c0=0, c1=C):
            dst = tile_[:].rearrange("p (c i m) -> p c i m", c=C, i=2)
            nc.sync.dma_start(out=dst[:, c0:c1], in_=src[fb, :, c0:c1])

        def load_w1_full(tiles, fb):
            for t_, s_ in zip(tiles, W_SRCS):
                w_dma(t_, s_, fb)

        # pass order inside a psum group: hi*hi, lo*hi, hi*lo -- the x_lo
        # tensor is only needed two-thirds into the group, easing the
        # pair-0 DMA-starved head
        def mm_group(psum, wh, wl, xh, xl):
            n = 3 * C
            k = 0
            for wt, xv in ((wh, xh), (wl, xh), (wh, xl)):
                for c in range(C):
                    nc.tensor.matmul(psum[:], wt[:, c], xv[:, c],
                                     start=(k == 0), stop=(k == n - 1),
                                     perf_mode=DR)
                    k += 1

        w_next = None
        xt_next = None
        for pr in range(NT // 2):
            t0, t1 = 2 * pr, 2 * pr + 1
            if pr == 0:
                # staged fine-grained first loads, priority-ordered for the
                # shared DMA pipe: x_hi[t0], w1[fb0], x_lo[t0], v1[fb0],
                # x[t1], w[fb1]
                xt = {t0: alloc_x(), t1: alloc_x()}
                w_first = {0: alloc_w1(), 1: alloc_w1()}
                bounds = [0, 1, 2, 4, 8, 12, C]
                for c0, c1 in zip(bounds, bounds[1:]):
                    x_dma(xt[t0][0], xh_d, t0, c0, c1)
                wb = [0, 1, 2, 4, 8, 12, C]
                for c0, c1 in zip(wb, wb[1:]):
                    w_dma(w_first[0][0], w1h_d, 0, c0, c1)
                    w_dma(w_first[0][1], w1l_d, 0, c0, c1)
                for c0, c1 in zip(bounds, bounds[1:]):
                    x_dma(xt[t0][1], xl_d, t0, c0, c1)
                for c0 in range(0, C, 8):
                    w_dma(w_first[0][2], v1h_d, 0, c0, c0 + 8)
                    w_dma(w_first[0][3], v1l_d, 0, c0, c0 + 8)
                for c0 in range(0, C, 4):
                    x_dma(xt[t1][0], xh_d, t1, c0, c0 + 4)
                load_w1_full(w_first[1], 1)
                for c0 in range(0, C, 4):
                    x_dma(xt[t1][1], xl_d, t1, c0, c0 + 4)
            else:
                xt = {t0: xt_next[0], t1: xt_next[1]}
                w_first = {0: w_next[0]}
                load_w1_full(w_next[1], 1)
                w_first[1] = w_next[1]

            ihi = {tt: ihi_pool.tile([128, FBN * TC], FP8, name="ihit")
                   for tt in (t0, t1)}
            ilo = {tt: ilo_pool.tile([128, FBN * TC], FP8, name="ilot")
                   for tt in (t0, t1)}

            # phase A: gate/up -> inter (hi/lo e4m3), per 128-row f-block
            for fb in range(FBN):
                if fb in w_first:
                    w1h_t, w1l_t, v1h_t, v1l_t = w_first[fb]
                else:
                    tiles = alloc_w1()
                    load_w1_full(tiles, fb)
                    w1h_t, w1l_t, v1h_t, v1l_t = tiles
                w1hv = w1h_t[:].rearrange("p (c i m) -> p c i m", c=C, i=2)
                w1lv = w1l_t[:].rearrange("p (c i m) -> p c i m", c=C, i=2)
                v1hv = v1h_t[:].rearrange("p (c i m) -> p c i m", c=C, i=2)
                v1lv = v1l_t[:].rearrange("p (c i m) -> p c i m", c=C, i=2)
                for tt in (t0, t1):
                    xhv = xt[tt][0][:].rearrange("p (c i t) -> p c i t", c=C, i=2)
                    xlv = xt[tt][1][:].rearrange("p (c i t) -> p c i t", c=C, i=2)
                    pg = pg_pool.tile([128, TC], F32)
                    pu = pu_pool.tile([128, TC], F32)
                    mm_group(pg, w1hv, w1lv, xhv, xlv)
                    mm_group(pu, v1hv, v1lv, xhv, xlv)
                    sg = sg_pool.tile([128, TC], F32)
                    nc.scalar.activation(sg[:], pg[:], AF.Sigmoid, scale=SIG_SCALE)
                    sl = sl_pool.tile([128, TC], F32)
                    nc.vector.tensor_mul(sl[:], sg[:], pg[:])
                    raw = raw_pool.tile([128, TC], F32)
                    nc.vector.tensor_mul(raw[:], sl[:], pu[:])
                    hi_sl = ihi[tt][:, bass.ts(fb, TC)]
                    nc.scalar.activation(hi_sl, raw[:], AF.Copy, scale=CAST_SCALE)
                    nc.vector.scalar_tensor_tensor(
                        ilo[tt][:, bass.ts(fb, TC)], raw[:], CAST_SCALE, hi_sl,
                        op0=ALU.mult, op1=ALU.subtract)

            # next pair's first weights + x, interleaved into phase B's DMA
            # stream (shared-pipe FIFO: w2[hb] must stay just-in-time)
            last = pr == NT // 2 - 1
            prefetch = []
            if not last:
                w_next = {0: alloc_w1(), 1: alloc_w1()}
                xt_next = (alloc_x(), alloc_x())
                na, nb = 2 * pr + 2, 2 * pr + 3
                for t_, s_ in zip(w_next[0], W_SRCS):
                    prefetch.append(lambda t_=t_, s_=s_: w_dma(t_, s_, 0))
                for q in range(0, C, 2):
                    prefetch.append(lambda q=q: x_dma(xt_next[0][0], xh_d, na, q, q + 2))
                    prefetch.append(lambda q=q: x_dma(xt_next[0][1], xl_d, na, q, q + 2))
                    prefetch.append(lambda q=q: x_dma(xt_next[1][0], xh_d, nb, q, q + 2))
                    prefetch.append(lambda q=q: x_dma(xt_next[1][1], xl_d, nb, q, q + 2))

            # phase B: partial down, per 128-row h-block
            for hb in range(HB):
                w2h_t = w2h_pool.tile([128, JN * 2 * 128], FP8)
                w2l_t = w2l_pool.tile([128, JN * 2 * 128], FP8)
                nc.sync.dma_start(
                    out=w2h_t[:].rearrange("p (j i m) -> p j i m", j=JN, i=2),
                    in_=w2h_d[hb])
                nc.sync.dma_start(
                    out=w2l_t[:].rearrange("p (j i m) -> p j i m", j=JN, i=2),
                    in_=w2l_d[hb])
                if 2 <= hb < 27 and prefetch:
                    prefetch.pop(0)()
                w2hv = w2h_t[:].rearrange("p (j i m) -> p j i m", j=JN, i=2)
                w2lv = w2l_t[:].rearrange("p (j i m) -> p j i m", j=JN, i=2)
                for tt in (t0, t1):
                    ihv = ihi[tt][:].rearrange("p (fb t) -> p fb t", fb=FBN)
                    ilv = ilo[tt][:].rearrange("p (fb t) -> p fb t", fb=FBN)
                    last_tile = last and hb == HB - 1 and tt == t1
                    splits = (0, 256, 384, 512) if last_tile else (0, TC)
                    for si in range(len(splits) - 1):
                        c0, c1 = splits[si], splits[si + 1]
                        pd = pd_pool.tile([128, c1 - c0], F32)
                        k = 0
                        for j in range(JN):
                            for wt, iv in ((w2hv, ihv), (w2lv, ihv), (w2hv, ilv)):
                                nc.tensor.matmul(
                                    pd[:], wt[:, j],
                                    iv[:, 2 * j:2 * j + 2, c0:c1],
                                    start=(k == 0), stop=(k == 3 * JN - 1),
                                    perf_mode=DR)
                                k += 1
                        ob = out_pool.tile([128, c1 - c0], BF16)
                        nc.scalar.activation(ob[:], pd[:], AF.Copy, scale=OUT_SCALE)
                        nc.sync.dma_start(
                            out=out_d[hb * 128:(hb + 1) * 128,
                                      tt * TC + c0:tt * TC + c1],
                            in_=ob[:])
            while prefetch:
                prefetch.pop(0)()

    nc.compile()
    return nc


def _split(a):
    hi = a.astype(E4)
    lo = (a - hi.astype(np.float32)).astype(E4)
    return np.ascontiguousarray(hi), np.ascontiguousarray(lo)


def _prep_inputs(x, w1, v1, w2):
    # x[t, h]*SX -> [tci, p(h%128), c(h//256), i((h%256)//128), tt]
    x4 = (x * SX).reshape(NT, TC, C, 2, 128).transpose(0, 4, 2, 3, 1)
    xh, xl = _split(np.ascontiguousarray(x4, dtype=np.float32))
    in_maps = []
    for cid in range(N_CORES):
        sl_ = slice(cid * FS, (cid + 1) * FS)
        # w[f, h]*S -> [fb, p(h%128), c, i, m(f%128)]
        w1s = (w1[sl_] * S1).reshape(FBN, 128, C, 2, 128).transpose(0, 4, 2, 3, 1)
        v1s = (v1[sl_] * SV).reshape(FBN, 128, C, 2, 128).transpose(0, 4, 2, 3, 1)
        # w2[f, h]*S2 -> [hb, p(f%128), j(f//256), i((f%256)//128), m(h%128)]
        w2s = (w2[sl_] * S2).reshape(JN, 2, 128, HB, 128).transpose(3, 2, 0, 1, 4)
        w1h, w1l = _split(np.ascontiguousarray(w1s, dtype=np.float32))
        v1h, v1l = _split(np.ascontiguousarray(v1s, dtype=np.float32))
        w2h, w2l = _split(np.ascontiguousarray(w2s, dtype=np.float32))
        in_maps.append({
            "xh": xh, "xl": xl,
            "w1h": w1h, "w1l": w1l, "v1h": v1h, "v1l": v1l,
            "w2h": w2h, "w2l": w2l,
        })
    return in_maps


def _exec_once(in_maps):
    """One 8-core device execution; returns summed partial [H, T] f32."""
    if not _NC_CACHE:
        _NC_CACHE.append(_build())
    res = run_bass_kernel_spmd(_NC_CACHE[0], in_maps, list(range(N_CORES)))
    acc = res.results[0]["out"].astype(np.float32)
    for c in range(1, N_CORES):
        acc += res.results[c]["out"].astype(np.float32)
    if not np.isfinite(acc).all():
        raise FloatingPointError("non-finite output from device")
    return acc


_IN_KEYS = ("xh", "xl", "w1h", "w1l", "v1h", "v1l", "w2h", "w2l")


def _exec_subprocess(in_maps):
    """Retry path: run the device execution in a fresh process (fresh axon
    client) in case this process's device session is poisoned."""
    base = "/dev/shm" if os.path.isdir("/dev/shm") else None
    with tempfile.TemporaryDirectory(dir=base) as d:
        np.save(os.path.join(d, "xh.npy"), in_maps[0]["xh"].view(np.uint8))
        np.save(os.path.join(d, "xl.npy"), in_maps[0]["xl"].view(np.uint8))
        for c, m in enumerate(in_maps):
            for k in _IN_KEYS[2:]:
                np.save(os.path.join(d, f"{k}_{c}.npy"), m[k].view(np.uint8))
        subprocess.run(
            [sys.executable, os.path.abspath(__file__), "--subproc", d],
            check=True, timeout=1800,
        )
        return np.load(os.path.join(d, "acc.npy"))


def _subproc_main(d):
    xh = np.load(os.path.join(d, "xh.npy")).view(E4)
    xl = np.load(os.path.join(d, "xl.npy")).view(E4)
    in_maps = []
    for c in range(N_CORES):
        m = {"xh": xh, "xl": xl}
        for k in _IN_KEYS[2:]:
            m[k] = np.load(os.path.join(d, f"{k}_{c}.npy")).view(E4)
        in_maps.append(m)
    np.save(os.path.join(d, "acc.npy"), _exec_once(in_maps))


def kernel(x, expert_w1, expert_v1, expert_w2):
    x = np.asarray(x, dtype=np.float32)
    expert_w1 = np.asarray(expert_w1, dtype=np.float32)
    expert_v1 = np.asarray(expert_v1, dtype=np.float32)
    expert_w2 = np.asarray(expert_w2, dtype=np.float32)
    assert x.shape == (T, H) and expert_w1.shape == (F, H)

    in_maps = _prep_inputs(x, expert_w1, expert_v1, expert_w2)

    acc = None
    last_err = None
    for attempt in range(4):
        try:
            if attempt < 2:
                acc = _exec_once(in_maps)
            else:
                acc = _exec_subprocess(in_maps)
            break
        except Exception as e:  # transient device/tunnel errors: retry
            last_err = e
            time.sleep(3.0)
    if acc is None:
        raise last_err
    return np.ascontiguousarray(acc.T)  # [h, t] -> [t, h]


if __name__ == "__main__" and len(sys.argv) == 3 and sys.argv[1] == "--subproc":
    _subproc_main(sys.argv[2])
